# revision 7
# baseline (speedup 1.0000x reference)
"""AnchorLoss distributed Trainium2 kernel (8 NeuronCores).

reference math (anchors: [8192, 8, 512] f32):
    x = anchors.reshape(8192, 4096)
    loss = -(2*N*sum(x*x) - 2*sum(colsum(x)^2)) / sqrt(512)

Strategy: shard rows (n_classes) across 8 cores. Each core streams its
[1024, 4096] shard from HBM in [128, 4096] tiles and computes
  - per-partition partial sum of squares (ScalarE Square-accumulate /
    VectorE tensor_tensor_reduce, alternating engines)
  - partial column sums via PE matmul (lhsT = x chunk [128,128],
    rhs = ones [128,1]) accumulated in one PSUM bank as [128, 32]
then AllReduces a packed [4224] f32 vector (colsum [4096] + per-partition
sumsq [128]) across the 8 cores and finishes the scalar identity on
every core. Host takes core 0's scalar.
"""

import numpy as np

from concourse import bass, bacc, tile, mybir
from concourse.bass_utils import run_bass_kernel_spmd

N_CORES = 8
N_CLASSES = 8192
D = 4096                       # 8 * 512 flattened embedding dim
ROWS = N_CLASSES // N_CORES    # 1024 rows per core
P = 128                        # partitions
N_TILES = ROWS // P            # 8 row tiles per core
CHUNK = 128                    # columns per colsum matmul
N_CHUNKS = D // CHUNK          # 32
FACTOR = float(np.sqrt(np.float32(512.0)))


def _build():
    nc = bacc.Bacc(None, num_devices=N_CORES)
    x_ext = nc.declare_dram_parameter(
        "anchors", [ROWS, D], mybir.dt.float32, isOutput=False
    )
    out_ext = nc.declare_dram_parameter(
        "out", [1, 1], mybir.dt.float32, isOutput=True
    )

    with tile.TileContext(nc) as tc:
        with (
            tc.tile_pool(name="io", bufs=4) as io,
            tc.tile_pool(name="small", bufs=1) as sp,
            tc.tile_pool(name="psum", bufs=1, space="PSUM") as ps,
            tc.tile_pool(name="dram", bufs=1, space="DRAM") as dr,
        ):
            ones = sp.tile([P, 1], mybir.dt.float32)
            nc.gpsimd.memset(ones[:], 1.0)
            rowsumsq = sp.tile([P, N_TILES], mybir.dt.float32)
            scr_s = sp.tile([P, D], mybir.dt.float32)
            cs_acc = sp.tile([P, N_CHUNKS], mybir.dt.float32)
            nc.vector.memset(cs_acc[:], 0.0)

            for t in range(N_TILES):
                xt = io.tile([P, D], mybir.dt.float32, tag="xt", name=f"xt{t}")
                nc.sync.dma_start(xt[:], x_ext[t * P:(t + 1) * P, :])
                # partial sum-of-squares on ScalarE (VectorE
                # tensor_tensor_reduce at FD=4096 crashes this runtime)
                nc.scalar.activation(
                    scr_s[:], xt[:],
                    mybir.ActivationFunctionType.Square,
                    accum_out=rowsumsq[:, t:t + 1],
                )
                # this tile's column sums: cs_ps[m, c] = sum_k xt[k, c*128+m]
                cs_ps = ps.tile(
                    [P, N_CHUNKS], mybir.dt.float32, tag="cs_ps", name=f"cs{t}",
                    bufs=2,
                )
                for c in range(N_CHUNKS):
                    nc.tensor.matmul(
                        cs_ps[:, c:c + 1],
                        lhsT=xt[:, c * CHUNK:(c + 1) * CHUNK],
                        rhs=ones[:],
                        start=True, stop=True,
                    )
                nc.vector.tensor_add(cs_acc[:], cs_acc[:], cs_ps[:])

            # local epilogue: pack [colsum(4096), per-partition sumsq(128)]
            rss = sp.tile([P, 1], mybir.dt.float32)
            nc.vector.tensor_reduce(
                out=rss[:], in_=rowsumsq[:],
                axis=mybir.AxisListType.X, op=mybir.AluOpType.add,
            )
            cc_in = dr.tile([D + P], mybir.dt.float32)
            cc_out = dr.tile([D + P], mybir.dt.float32, addr_space="Shared")
            nc.sync.dma_start(cc_in[0:D], cs_acc[:])
            nc.sync.dma_start(cc_in[D:D + P], rss[:])
            nc.gpsimd.collective_compute(
                "AllReduce",
                mybir.AluOpType.add,
                replica_groups=[list(range(N_CORES))],
                ins=[cc_in[:]],
                outs=[cc_out[:]],
            )

            # finish the identity on every core
            sq_in = sp.tile([P, N_CHUNKS], mybir.dt.float32)
            F = sp.tile([P, 2], mybir.dt.float32)
            nc.sync.dma_start(sq_in[:], cc_out[0:D])
            nc.sync.dma_start(F[:, 0:1], cc_out[D:D + P])
            scr2 = sp.tile([P, N_CHUNKS], mybir.dt.float32)
            nc.scalar.activation(
                scr2[:], sq_in[:],
                mybir.ActivationFunctionType.Square,
                accum_out=F[:, 1:2],
            )
            res_ps = ps.tile([1, 2], mybir.dt.float32)
            nc.tensor.matmul(res_ps[:], lhsT=ones[:], rhs=F[:], start=True, stop=True)
            # loss = (2/f)*colsumsq - (2*N/f)*sumsq
            a_sb = sp.tile([1, 1], mybir.dt.float32)
            nc.vector.tensor_scalar_mul(
                a_sb[:], res_ps[0:1, 0:1], float(2.0 * N_CLASSES / FACTOR)
            )
            loss_sb = sp.tile([1, 1], mybir.dt.float32)
            nc.vector.scalar_tensor_tensor(
                out=loss_sb[:], in0=res_ps[0:1, 1:2],
                scalar=float(2.0 / FACTOR), in1=a_sb[:],
                op0=mybir.AluOpType.mult, op1=mybir.AluOpType.subtract,
            )
            nc.sync.dma_start(out_ext[:], loss_sb[:])
    nc.finalize()
    return nc


_NC_CACHE = None


def _get_nc():
    global _NC_CACHE
    if _NC_CACHE is None:
        _NC_CACHE = _build()
    return _NC_CACHE


def _run(anchors: np.ndarray, trace: bool = False):
    """Returns (loss_scalar, BassKernelResults)."""
    x = np.ascontiguousarray(
        np.asarray(anchors, dtype=np.float32).reshape(N_CLASSES, D)
    )
    in_maps = [
        {"anchors": x[i * ROWS:(i + 1) * ROWS]} for i in range(N_CORES)
    ]
    nc = _get_nc()
    res = run_bass_kernel_spmd(nc, in_maps, core_ids=list(range(N_CORES)), trace=trace)
    loss = np.float32(np.asarray(res.results[0]["out"]).reshape(())[()])
    return loss, res


def kernel(anchors: np.ndarray) -> np.ndarray:
    loss, _ = _run(anchors)
    return np.asarray(loss, dtype=np.float32).reshape(())


# revision 20
# speedup vs baseline: 133.3651x; 133.3651x over previous
"""AnchorLoss distributed Trainium2 kernel (8 NeuronCores).

reference math (anchors: [8192, 8, 512] f32):
    x = anchors.reshape(8192, 4096)
    loss = -(2*N*sum(x*x) - 2*sum(colsum(x)^2)) / sqrt(512)

Strategy: shard COLUMNS across the 8 cores (512 columns each). Each core
streams its [8192, 512] column slice (16 MiB) in 16 tiles of
[128, 4x512] (4 row-blocks per tile) and computes
  - per-partition partial sum of squares (ScalarE Square-accumulate)
  - the COMPLETE column sums of its 512 columns via PE matmuls
    (lhsT = x block [128,128], rhs = ones [128,1], PSUM-accumulated
    over the 4 row-blocks of the tile, then SBUF-accumulated over tiles)
so the only cross-core data is one scalar per core:
    c_k = (2/f)*||colsum_k||^2 - (2*N/f)*sumsq_k
AllGather the 8 scalars, every core sums them -> loss (= -total/f).
Host takes core 0's scalar.
"""

import numpy as np

from concourse import bacc, tile, mybir
from concourse.bass_utils import run_bass_kernel_spmd

N_CORES = 8
N_CLASSES = 8192
D = 4096                        # 8 * 512 flattened embedding dim
COLS = D // N_CORES             # 512 columns per core
P = 128                         # partitions
RB = 4                          # row-blocks per tile
TILE_ROWS = P * RB              # 512 rows per tile
N_TILES = N_CLASSES // TILE_ROWS  # 16
CHUNK = 128                     # columns per colsum matmul
N_CHUNKS = COLS // CHUNK        # 4
FACTOR = float(np.sqrt(np.float32(512.0)))


def _build():
    nc = bacc.Bacc(None, num_devices=N_CORES)
    x_ext = nc.declare_dram_parameter(
        "anchors", [N_CLASSES, COLS], mybir.dt.float32, isOutput=False
    )
    out_ext = nc.declare_dram_parameter(
        "out", [1, 1], mybir.dt.float32, isOutput=True
    )

    with tile.TileContext(nc) as tc:
        with (
            tc.tile_pool(name="io", bufs=4) as io,
            tc.tile_pool(name="small", bufs=1) as sp,
            tc.tile_pool(name="psum", bufs=1, space="PSUM") as ps,
            tc.tile_pool(name="dram", bufs=1, space="DRAM") as dr,
        ):
            ones = sp.tile([P, 1], mybir.dt.float32)
            nc.gpsimd.memset(ones[:], 1.0)
            # one accum column per (tile, sub-square): the last two tiles
            # split their square into RB chunks to shorten the critical tail
            rowsumsq = sp.tile([P, N_TILES + 2 * (RB - 1)], mybir.dt.float32)
            scr_s = sp.tile([P, RB, COLS], mybir.dt.float32)
            scr_v = sp.tile([P, COLS], mybir.dt.float32)
            cs_acc = sp.tile([P, N_CHUNKS], mybir.dt.float32)
            nc.vector.memset(cs_acc[:], 0.0)

            for t in range(N_TILES):
                xt = io.tile([P, RB, COLS], mybir.dt.float32, tag="xt",
                             name=f"xt{t}")
                src = x_ext[t * TILE_ROWS:(t + 1) * TILE_ROWS, :]
                src = src.rearrange("(rb p) c -> p rb c", rb=RB, p=P)
                # the last two tiles are DMA'd and squared per row-block so
                # only a short square trails the final DMA
                if t < N_TILES - 2:
                    nc.sync.dma_start(xt[:], src)
                    nc.scalar.activation(
                        scr_s[:], xt[:],
                        mybir.ActivationFunctionType.Square,
                        accum_out=rowsumsq[:, t:t + 1],
                    )
                else:
                    base = t + (t - (N_TILES - 2)) * (RB - 1)
                    for j in range(RB):
                        nc.sync.dma_start(xt[:, j, :], src[:, j, :])
                        col = rowsumsq[:, base + j:base + j + 1]
                        if j == 1:
                            # keep ScalarE's queue short near the tail
                            nc.vector.tensor_mul(scr_v[:], xt[:, j, :],
                                                 xt[:, j, :])
                            nc.vector.tensor_reduce(
                                out=col, in_=scr_v[:],
                                axis=mybir.AxisListType.X,
                                op=mybir.AluOpType.add,
                            )
                        else:
                            nc.scalar.activation(
                                scr_s[:, j, :], xt[:, j, :],
                                mybir.ActivationFunctionType.Square,
                                accum_out=col,
                            )
                # column sums of this tile's 512 rows:
                # cs_ps[m, c] = sum_{rb,p} xt[p, rb, c*128+m]
                cs_ps = ps.tile(
                    [P, N_CHUNKS], mybir.dt.float32, tag="cs_ps",
                    name=f"cs{t}", bufs=2,
                )
                for c in range(N_CHUNKS):
                    for j in range(RB):
                        nc.tensor.matmul(
                            cs_ps[:, c:c + 1],
                            lhsT=xt[:, j, c * CHUNK:(c + 1) * CHUNK],
                            rhs=ones[:],
                            start=(j == 0), stop=(j == RB - 1),
                        )
                nc.vector.tensor_add(cs_acc[:], cs_acc[:], cs_ps[:])

            # local scalars: F[:,0] = per-partition sumsq, F[:,1] = colsum^2
            F = sp.tile([P, 2], mybir.dt.float32)
            nc.vector.tensor_reduce(
                out=F[:, 0:1], in_=rowsumsq[:],
                axis=mybir.AxisListType.X, op=mybir.AluOpType.add,
            )
            # colsum^2 on DVE (keeps it off ScalarE's tail queue)
            scr2 = sp.tile([P, N_CHUNKS], mybir.dt.float32)
            nc.vector.tensor_mul(scr2[:], cs_acc[:], cs_acc[:])
            nc.vector.tensor_reduce(
                out=F[:, 1:2], in_=scr2[:],
                axis=mybir.AxisListType.X, op=mybir.AluOpType.add,
            )
            res_ps = ps.tile([1, 2], mybir.dt.float32)
            nc.tensor.matmul(res_ps[:], lhsT=ones[:], rhs=F[:],
                             start=True, stop=True)
            # c_k = (2/f)*colsumsq_k - (2*N/f)*sumsq_k
            a_sb = sp.tile([1, 1], mybir.dt.float32)
            nc.vector.tensor_scalar_mul(
                a_sb[:], res_ps[0:1, 0:1], float(2.0 * N_CLASSES / FACTOR)
            )
            ck_sb = sp.tile([1, 1], mybir.dt.float32)
            nc.vector.scalar_tensor_tensor(
                out=ck_sb[:], in0=res_ps[0:1, 1:2],
                scalar=float(2.0 / FACTOR), in1=a_sb[:],
                op0=mybir.AluOpType.mult, op1=mybir.AluOpType.subtract,
            )

            # sum the 8 per-core scalars: replicate ck 8x, ReduceScatter-add
            # -> each core's [1] output IS the loss; copy DRAM->DRAM to out
            ck8 = sp.tile([1, N_CORES], mybir.dt.float32)
            nc.vector.tensor_copy(ck8[:], ck_sb[:].broadcast_to([1, N_CORES]))
            cc_in = dr.tile([N_CORES], mybir.dt.float32)
            cc_out = dr.tile([1], mybir.dt.float32)
            nc.sync.dma_start(cc_in[:], ck8[:])
            nc.gpsimd.collective_compute(
                "ReduceScatter",
                mybir.AluOpType.add,
                replica_groups=[list(range(N_CORES))],
                ins=[cc_in[:]],
                outs=[cc_out[:]],
            )
            nc.sync.dma_start(out_ext[:], cc_out[:])
    nc.finalize()
    return nc


_NC_CACHE = None


def _get_nc():
    global _NC_CACHE
    if _NC_CACHE is None:
        _NC_CACHE = _build()
    return _NC_CACHE


def _run(anchors: np.ndarray, trace: bool = False):
    """Returns (loss_scalar, BassKernelResults)."""
    x = np.asarray(anchors, dtype=np.float32).reshape(N_CLASSES, D)
    in_maps = [
        {"anchors": np.ascontiguousarray(x[:, i * COLS:(i + 1) * COLS])}
        for i in range(N_CORES)
    ]
    nc = _get_nc()
    res = run_bass_kernel_spmd(nc, in_maps, core_ids=list(range(N_CORES)), trace=trace)
    loss = np.float32(np.asarray(res.results[0]["out"]).reshape(())[()])
    return loss, res


def kernel(anchors: np.ndarray) -> np.ndarray:
    loss, _ = _run(anchors)
    return np.asarray(loss, dtype=np.float32).reshape(())
